# revision 15
# baseline (speedup 1.0000x reference)
"""Trainium2 Bass kernel for LocalMQA (windowed multi-head attention block).

Data-parallel over (batch, sequence): each of 8 cores owns 1024 consecutive
query tokens (2 buckets of W=512) of one batch element, plus a 512-token
halo for K/V.  No collectives (per-call collective setup costs ~1.5ms under
this runtime — measured).

Per-call wire traffic dominates the metric under the axon tunnel, so the
input is shipped as INT8 with one fp32 scale per token column: the qk
l2-normalization makes q/k scale-invariant per token, v is un-scaled for
free inside its PSUM-drain copy (tensor_scalar by the per-token scale on the
partition axis), and only the sigmoid gate logits need an explicit unscale
(PE broadcast of the reciprocal scales).  Weights, scales and band masks are
baked into the NEFF as Consts.  Output returns as bf16.

Per-core pipeline (bf16 matmuls, fp32 PSUM):
  1. k/v projections over the 1536-token extended range (int8 input is cast
     to bf16 during the SWDGE DMA); l2norm of k via ACT Square + PE
     ones-matmul + ACT Sqrt + DVE reciprocal + PE outer broadcast.
  2. q projection with the same normalization (q_scale*SCALE folded in);
     sigmoid gates with reciprocal-scale correction; gate rows staged to
     partition 0 by 8 small SBUF->SBUF DMAs.
  3. Windowed attention computed transposed (simT[j,i] = k_j.q_i) and
     BANDED: per window chunk jc only the i-columns that can be valid are
     computed (62.5% of the full rectangle); only the diagonal 128x128 block
     needs a mask multiply (triangular consts), except bucket 0's halo
     chunks which use a full-width mask pre-multiplied by the halo-valid
     flag.  Softmax without max-subtraction (|sim| <= 8); denominator via PE
     ones-matmul.
  4. Output projection accumulating over heads into [128, 2048] staging
     tiles, one DMA per 128-token block.
"""

import hashlib
import sys

import numpy as np
import ml_dtypes

try:
    import concourse.bass as bass  # noqa: F401
except ImportError:  # pragma: no cover
    sys.path.insert(0, "/opt/trn_rl_repo")

import concourse.bass as bass
import concourse.tile as tile
from concourse import bacc, mybir

BF = ml_dtypes.bfloat16
B, N, D = 2, 4096, 2048
H, DH, W = 8, 128, 512
SCALE = 8.0
NCORES = 8
TOK = (B * N) // NCORES          # 1024 own tokens per core
EXT = TOK + W                    # 1536 tokens incl. halo
DC = D // 128                    # 16 d-chunks
NBL = TOK // W                   # 2 buckets per core
BFD = mybir.dt.bfloat16
F32 = mybir.dt.float32
I8 = mybir.dt.int8

# banded-attention geometry: per window chunk jc, the computed query-column
# range [IOFF, IOFF+WID) and the start of the in-slice diagonal block
IOFF = [0, 0, 0, 0, 0, 128, 256, 384]
WID = [128, 256, 384, 512, 512, 384, 256, 128]
DIAG = [0, 128, 256, 384, 0, 0, 0, 0]
B0OFF = [0, 128, 384, 768]       # offsets of the bl=0 full masks in cB0


def _r128(ap):
    """(K, F) dram AP -> (128, K//128, F) partition-major view."""
    return ap.rearrange("(po pi) f -> pi po f", pi=128)


def _const_arrays(Wq, Wkv, q_scale, k_scale, Wg, bg, Wo):
    wqt = np.ascontiguousarray(np.asarray(Wq, np.float32).T).astype(BF)
    wkt = np.ascontiguousarray(
        np.asarray(Wkv[:H * DH], np.float32).T).astype(BF)
    wvt = np.ascontiguousarray(
        np.asarray(Wkv[H * DH:], np.float32).T).astype(BF)
    wgt = np.ascontiguousarray(np.asarray(Wg, np.float32).T).astype(BF)
    wot = np.ascontiguousarray(np.asarray(Wo, np.float32).T).astype(BF)
    qs = (np.asarray(q_scale, np.float32) * SCALE).reshape(1, DH).astype(BF)
    ks = np.asarray(k_scale, np.float32).reshape(1, DH).astype(BF)
    bgc = np.ascontiguousarray(np.asarray(bg, np.float32).reshape(H, 1))

    ii = np.arange(128)[None, :]
    jj = np.arange(128)[:, None]
    mL = (ii <= jj).astype(BF)                      # prev-bucket diag block
    mU = (ii >= jj).astype(BF)                      # own-bucket diag block
    b0 = np.zeros((128, 1280), np.float32)
    for jc in range(4):
        w = WID[jc]
        blk = np.ones((128, w), np.float32)
        blk[:, 128 * jc:] = (ii <= jj).astype(np.float32)
        b0[:, B0OFF[jc]:B0OFF[jc] + w] = blk
    return {
        "cwqt": wqt, "cwkt": wkt, "cwvt": wvt, "cwgt": wgt, "cwot": wot,
        "cqs": qs, "cks": ks, "conesc": np.ones((128, 1), BF),
        "conesr": np.ones((1, 128), BF), "conesh": np.ones((1, H), BF),
        "cbg": bgc,
        "cmL": np.ascontiguousarray(mL), "cmU": np.ascontiguousarray(mU),
        "cB0": np.ascontiguousarray(b0.astype(BF)),
    }


def build_nc(Wq=None, Wkv=None, q_scale=None, k_scale=None, Wg=None,
             bg=None, Wo=None, **_ignored):
    consts = _const_arrays(Wq, Wkv, q_scale, k_scale, Wg, bg, Wo)

    nc = bacc.Bacc("TRN2", target_bir_lowering=False, debug=False,
                   num_devices=NCORES)

    xz_d = nc.dram_tensor("xz", (D, EXT), I8, kind="ExternalInput").ap()
    sc_d = nc.dram_tensor("sc", (128, 16), F32, kind="ExternalInput").ap()
    sg_d = nc.dram_tensor("sg", (1, TOK), BFD, kind="ExternalInput").ap()
    y_d = nc.dram_tensor("y", (TOK, D), BFD, kind="ExternalOutput").ap()

    cap = {k: nc.inline_tensor(v, name=k).ap() for k, v in consts.items()}

    with tile.TileContext(nc) as tc:
        _emit(tc, nc, xz_d, sc_d, sg_d, cap, y_d)
    nc.compile()
    return nc, consts


def _emit(tc, nc, xz_d, sc_d, sg_d, cap, y_d):
    Exp = mybir.ActivationFunctionType.Exp
    Sqrt = mybir.ActivationFunctionType.Sqrt
    Sigmoid = mybir.ActivationFunctionType.Sigmoid
    Square = mybir.ActivationFunctionType.Square
    MUL = mybir.AluOpType.mult

    from contextlib import ExitStack
    ctx = ExitStack()
    with ctx:
        persist = ctx.enter_context(tc.tile_pool(name="persist", bufs=1))
        wpool = ctx.enter_context(tc.tile_pool(name="wpool", bufs=2))
        scr = ctx.enter_context(tc.tile_pool(name="scr", bufs=3))

        # ---- persistent tiles -------------------------------------------
        kT = persist.tile([128, H, EXT], BFD)        # [dh, h, ext_t]
        vS = persist.tile([128, EXT // 128, H * DH], BFD)  # [t%128, tblk, c]
        qT = persist.tile([128, H, TOK], BFD)        # [dh, h, own_t]
        gT = persist.tile([H, TOK], BFD)             # gates [h, own_t]
        gRow = persist.tile([1, H * TOK], BFD)       # gates on partition 0
        stok = persist.tile([128, 16], F32, tag="c_st")   # per-token scales
        sgi = persist.tile([1, TOK], BFD, tag="c_sg")     # s own tokens
        qs_t = persist.tile([1, DH], BFD, tag="c_qs")
        ks_t = persist.tile([1, DH], BFD, tag="c_ks")
        ones_c = persist.tile([128, 1], BFD, tag="c_oc")
        ones_r = persist.tile([1, 128], BFD, tag="c_or")
        ones_h = persist.tile([1, H], BFD, tag="c_oh")
        bg_t = persist.tile([H, 1], F32, tag="c_bg")
        wg_t = persist.tile([128, DC, H], BFD, tag="c_wg")
        mL = persist.tile([128, 128], BFD, tag="c_mL")
        mU = persist.tile([128, 128], BFD, tag="c_mU")
        cB0 = persist.tile([128, 1280], BFD, tag="c_B0")
        mask0 = persist.tile([128, 1280], BFD, tag="mask0")
        eps_t = persist.tile([1, 1], F32, tag="c_eps")
        nc.gpsimd.memset(eps_t[:], 1e-12)
        nc.sync.dma_start(stok[:], sc_d[:])
        nc.sync.dma_start(sgi[:], sg_d[:])
        nc.sync.dma_start(qs_t[:], cap["cqs"][:])
        nc.sync.dma_start(ks_t[:], cap["cks"][:])
        nc.sync.dma_start(ones_c[:], cap["conesc"][:])
        nc.sync.dma_start(ones_r[:], cap["conesr"][:])
        nc.sync.dma_start(ones_h[:], cap["conesh"][:])
        nc.sync.dma_start(bg_t[:], cap["cbg"][:])
        nc.sync.dma_start(wg_t[:], _r128(cap["cwgt"]))
        nc.sync.dma_start(mL[:], cap["cmL"][:])
        nc.sync.dma_start(mU[:], cap["cmU"][:])
        nc.sync.dma_start(cB0[:], cap["cB0"][:])
        # bl=0 halo masks: band pattern times the halo-valid flag (sc col 12)
        nc.vector.tensor_scalar_mul(mask0[:], cB0[:], stok[:, 12:13])

        # ---- weight tiles: ring of 6 slots, each 4 d-chunks (8KB/part) --
        def load_w(capname):
            tiles = []
            for i in range(4):
                t = wpool.tile([128, 4, H * DH], BFD, tag="w", bufs=6)
                nc.sync.dma_start(t[:],
                                  _r128(cap[capname])[:, 4 * i:4 * i + 4, :])
                tiles.append(t)
            return tiles

        def wsl(tiles, dc, lo, size):
            return tiles[dc // 4][:, dc % 4, lo:lo + size]

        wk = load_w("cwkt")
        wv = load_w("cwvt")

        def norm_drain(ppsum, psum_tile, scale_row, out_slice):
            """l2norm columns of psum (dh, 512), scale, write bf16."""
            sq = scr.tile([128, 512], BFD, tag="sq")
            nc.scalar.activation(sq[:], psum_tile[:], Square)
            ssp = ppsum.tile([1, 512], F32, tag="pnarrow", bufs=2)
            nc.tensor.matmul(ssp[:], ones_c[:], sq[:], start=True, stop=True)
            srt = scr.tile([1, 512], F32, tag="srt", bufs=2)
            nc.scalar.activation(srt[:], ssp[:], Sqrt, bias=eps_t[:])
            nc.vector.reciprocal(srt[:], srt[:])
            rn = scr.tile([1, 512], BFD, tag="rn", bufs=2)
            nc.vector.tensor_copy(rn[:], srt[:])
            obp = ppsum.tile([128, 512], F32, tag="pouter", bufs=2)
            nc.tensor.matmul(obp[:], scale_row[:], rn[:], start=True,
                             stop=True)
            osb = scr.tile([128, 512], BFD, tag="osb")
            nc.vector.tensor_copy(osb[:], obp[:])
            nc.vector.tensor_tensor(out_slice, psum_tile[:], osb[:], MUL)

        with (tc.tile_pool(name="xpool", bufs=1) as xpool,
              tc.tile_pool(name="ppsum", bufs=1, space="PSUM") as ppsum):
            # int8 x is cast to bf16 during the (SWDGE) DMA
            xt = []
            for dc in range(DC):
                t = xpool.tile([128, EXT], BFD, tag="xt", bufs=DC,
                               name=f"xt{dc}")
                nc.gpsimd.dma_start(t[:], _r128(xz_d)[:, dc, :])
                xt.append(t)

            # ---- k projection + l2norm -----------------------------------
            for h in range(H):
                pks = [ppsum.tile([128, 512], F32, tag="pk", bufs=4,
                                  name=f"pk{h}_{t3}")
                       for t3 in range(3)]
                for dc in range(DC):
                    for t3 in range(3):
                        nc.tensor.matmul(
                            pks[t3][:], wsl(wk, dc, DH * h, DH),
                            xt[dc][:, 512 * t3:512 * (t3 + 1)],
                            start=(dc == 0), stop=(dc == DC - 1))
                for t3 in range(3):
                    norm_drain(ppsum, pks[t3], ks_t,
                               kT[:, h, 512 * t3:512 * (t3 + 1)])

            # ---- v projection (token-major, un-scale in the drain) -------
            for tb in range(EXT // 128):
                pvs = [ppsum.tile([128, 512], F32, tag="pk", bufs=4,
                                  name=f"pv{tb}_{i}")
                       for i in range(2)]
                for dc in range(DC):
                    for cb in range(2):
                        nc.tensor.matmul(
                            pvs[cb][:], xt[dc][:, 128 * tb:128 * (tb + 1)],
                            wsl(wv, dc, 512 * cb, 512),
                            start=(dc == 0), stop=(dc == DC - 1))
                for cb in range(2):
                    nc.vector.tensor_scalar_mul(
                        vS[:, tb, 512 * cb:512 * (cb + 1)], pvs[cb][:],
                        stok[:, tb:tb + 1])

            # ---- gates (with reciprocal-scale correction) ----------------
            for t2 in range(TOK // 512):
                pg = ppsum.tile([H, 512], F32, tag="pnarrow", bufs=2)
                for dc in range(DC):
                    nc.tensor.matmul(
                        pg[:], wg_t[:, dc, :],
                        xt[dc][:, W + 512 * t2:W + 512 * (t2 + 1)],
                        start=(dc == 0), stop=(dc == DC - 1))
                sop = ppsum.tile([H, 512], F32, tag="pouter", bufs=2)
                nc.tensor.matmul(sop[:], ones_h[:],
                                 sgi[0:1, 512 * t2:512 * (t2 + 1)],
                                 start=True, stop=True)
                sos = scr.tile([H, 512], BFD, tag="sos", bufs=2)
                nc.vector.tensor_copy(sos[:], sop[:])
                pgs = scr.tile([H, 512], F32, tag="pgs", bufs=2)
                nc.vector.tensor_tensor(pgs[:], pg[:], sos[:], MUL)
                nc.scalar.activation(gT[:, 512 * t2:512 * (t2 + 1)], pgs[:],
                                     Sigmoid, bias=bg_t[:])
            for h in range(H):
                nc.sync.dma_start(gRow[0:1, h * TOK:(h + 1) * TOK],
                                  gT[h:h + 1, :])

            # ---- q projection + l2norm (recycles ring slots) -------------
            wq = load_w("cwqt")
            for h in range(H):
                pqs = [ppsum.tile([128, 512], F32, tag="pk", bufs=4,
                                  name=f"pq{h}_{t2}")
                       for t2 in range(TOK // 512)]
                for dc in range(DC):
                    for t2 in range(TOK // 512):
                        nc.tensor.matmul(
                            pqs[t2][:], wsl(wq, dc, DH * h, DH),
                            xt[dc][:, W + 512 * t2:W + 512 * (t2 + 1)],
                            start=(dc == 0), stop=(dc == DC - 1))
                for t2 in range(TOK // 512):
                    norm_drain(ppsum, pqs[t2], qs_t,
                               qT[:, h, 512 * t2:512 * (t2 + 1)])

        # xpool closed: its SBUF is reused by the attention pool below.
        wot = []
        for i in range(4):
            t = wpool.tile([128, 2, D], BFD, tag="w", bufs=6)
            nc.sync.dma_start(t[:], _r128(cap["cwot"])[:, 2 * i:2 * i + 2, :])
            wot.append(t)

        with (tc.tile_pool(name="attn", bufs=1) as apool,
              tc.tile_pool(name="apsum", bufs=1, space="PSUM") as apsum):
            oT = apool.tile([128, H, TOK], BFD)       # [dh, h, own_t]

            for bl in (1, 0):
                for h in range(H):
                    pms = []
                    for jc in range(8):
                        w, io, dg = WID[jc], IOFF[jc], DIAG[jc]
                        sim = apsum.tile([128, 512], F32, tag="sim", bufs=2)
                        nc.tensor.matmul(
                            sim[:, :w],
                            kT[:, h, 512 * bl + 128 * jc:
                                     512 * bl + 128 * (jc + 1)],
                            qT[:, h, 512 * bl + io:512 * bl + io + w],
                            start=True, stop=True)
                        pm = apool.tile([128, 512], BFD, tag="pm", bufs=8)
                        nc.scalar.activation(pm[:, :w], sim[:, :w], Exp)
                        if bl == 0 and jc < 4:
                            nc.vector.tensor_tensor(
                                pm[:, :w], pm[:, :w],
                                mask0[:, B0OFF[jc]:B0OFF[jc] + w], MUL)
                        else:
                            mt = mL if jc < 4 else mU
                            nc.vector.tensor_tensor(
                                pm[:, dg:dg + 128], pm[:, dg:dg + 128],
                                mt[:], MUL)
                        pms.append(pm)
                    ops = apsum.tile([128, 512], F32, tag="po", bufs=2)
                    ssp = apsum.tile([1, 512], F32, tag="pss", bufs=2)
                    for jc in range(8):
                        w, io = WID[jc], IOFF[jc]
                        nc.tensor.matmul(
                            ops[:, io:io + w],
                            vS[:, 4 * bl + jc, DH * h:DH * (h + 1)],
                            pms[jc][:, :w], start=(jc == 0), stop=(jc == 7))
                        nc.tensor.matmul(
                            ssp[:, io:io + w], ones_c[:], pms[jc][:, :w],
                            start=(jc == 0), stop=(jc == 7))
                    rr = apool.tile([1, 512], F32, tag="rr", bufs=2)
                    nc.vector.reciprocal(rr[:], ssp[:])
                    rg = apool.tile([1, 512], BFD, tag="rg", bufs=2)
                    nc.vector.tensor_tensor(
                        rg[:], rr[:],
                        gRow[0:1, h * TOK + 512 * bl:h * TOK + 512 * bl + 512],
                        MUL)
                    rgp = apsum.tile([128, 512], F32, tag="prgb", bufs=1)
                    nc.tensor.matmul(rgp[:], ones_r[:], rg[:], start=True,
                                     stop=True)
                    rgb = apool.tile([128, 512], BFD, tag="rgb", bufs=2)
                    nc.vector.tensor_copy(rgb[:], rgp[:])
                    nc.vector.tensor_tensor(
                        oT[:, h, 512 * bl:512 * (bl + 1)], ops[:], rgb[:],
                        MUL)

                # ---- output projection for this bucket -------------------
                for tq in range(4):
                    tck = 4 * bl + tq
                    ysb = apool.tile([128, D], BFD, tag="ysb", bufs=2)
                    for do in range(4):
                        yp = apsum.tile([128, 512], F32, tag="py", bufs=1)
                        for h in range(H):
                            nc.tensor.matmul(
                                yp[:],
                                oT[:, h, 128 * tck:128 * (tck + 1)],
                                wot[h // 2][:, h % 2,
                                            512 * do:512 * (do + 1)],
                                start=(h == 0), stop=(h == H - 1))
                        nc.vector.tensor_copy(ysb[:, 512 * do:512 * (do + 1)],
                                              yp[:])
                    nc.sync.dma_start(_r128(y_d)[:, tck, :], ysb[:])


def make_core_inputs(x, **_ignored):
    """Host-side sharding, int8 quantization + layout prep."""
    x = np.asarray(x, np.float32)
    in_maps = []
    per_core = B * N // NCORES
    for c in range(NCORES):
        g0 = c * per_core
        b_idx, t0 = g0 // N, g0 % N
        lo = t0 - W
        xe = np.zeros((EXT, D), np.float32)
        s = max(lo, 0)
        xe[s - lo:] = x[b_idx, s:t0 + TOK]
        absmax = np.abs(xe).max(axis=1)
        absmax[absmax == 0.0] = 1.0
        scl = (absmax / 127.0).astype(np.float32)           # (EXT,)
        xq = np.clip(np.rint(xe / scl[:, None]), -127, 127).astype(np.int8)
        sc = np.zeros((128, 16), np.float32)
        sc[:, :12] = scl.reshape(12, 128).T
        sc[:, 12] = 0.0 if t0 == 0 else 1.0                 # halo valid
        sg = scl[W:].reshape(1, TOK).astype(BF)
        in_maps.append({"xz": np.ascontiguousarray(xq.T), "sc": sc,
                        "sg": sg})
    return in_maps


def make_runner(nc, in_maps):
    """Persistent jitted executor.

    Binds ONLY the real ExternalInputs as operands (outputs are allocated by
    PJRT, not shipped as pre-zeroed donated buffers — the kernel writes every
    output element, so zero-init is unnecessary and shipping the zero buffers
    per call costs ~1ms of wire time under the axon tunnel).
    """
    import jax
    from jax.sharding import Mesh, PartitionSpec
    try:
        from jax.experimental.shard_map import shard_map
    except ImportError:
        from jax.shard_map import shard_map
    from concourse.bass2jax import (_bass_exec_p, install_neuronx_cc_hook,
                                    partition_id_tensor)

    install_neuronx_cc_hook()
    partition_name = (nc.partition_id_tensor.name
                      if nc.partition_id_tensor else None)
    in_names, out_names, out_avals = [], [], []
    for alloc in nc.m.functions[0].allocations:
        if not isinstance(alloc, mybir.MemoryLocationSet):
            continue
        name = alloc.memorylocations[0].name
        if alloc.kind == "ExternalInput":
            if name != partition_name:
                in_names.append(name)
        elif alloc.kind == "ExternalOutput":
            out_names.append(name)
            out_avals.append(jax.core.ShapedArray(
                tuple(alloc.tensor_shape), mybir.dt.np(alloc.dtype)))
    n_params = len(in_names)
    all_names = list(in_names)
    if partition_name is not None:
        all_names.append(partition_name)

    def _body(*args):
        operands = list(args)
        if partition_name is not None:
            operands.append(partition_id_tensor())
        outs = _bass_exec_p.bind(
            *operands, out_avals=tuple(out_avals), in_names=tuple(all_names),
            out_names=tuple(out_names), lowering_input_output_aliases=(),
            sim_require_finite=False, sim_require_nnan=False, nc=nc)
        return tuple(outs)

    devices = jax.devices()[:NCORES]
    mesh = Mesh(np.asarray(devices), ("core",))
    run = jax.jit(
        shard_map(_body, mesh=mesh,
                  in_specs=(PartitionSpec("core"),) * n_params,
                  out_specs=(PartitionSpec("core"),) * len(out_names),
                  check_rep=False),
        keep_unused=True)
    concat_in = [np.concatenate([np.asarray(in_maps[c][nm])
                                 for c in range(NCORES)], axis=0)
                 for nm in in_names]
    args = [jax.device_put(a) for a in concat_in]
    return run, args, list(in_names)


def assemble_output(out_np):
    """out_np: list with the concatenated 'y' array -> full (B, N, D) f32."""
    y = out_np[0]
    out = np.empty((B, N, D), np.float32)
    per_core = B * N // NCORES
    for c in range(NCORES):
        g0 = c * per_core
        out[g0 // N, g0 % N:g0 % N + TOK] = \
            y[c * TOK:(c + 1) * TOK].astype(np.float32)
    return out


_NC_CACHE = None        # (weight_hash, nc, (run, order), args, weight_ids)


def _whash(inputs):
    h = hashlib.sha256()
    for k in ("Wq", "Wkv", "q_scale", "k_scale", "Wg", "bg", "Wo"):
        h.update(np.ascontiguousarray(np.asarray(inputs[k], np.float32)))
    return h.hexdigest()


def kernel(**inputs):
    global _NC_CACHE
    import jax
    wids = tuple(id(inputs[k]) for k in
                 ("Wq", "Wkv", "q_scale", "k_scale", "Wg", "bg", "Wo"))
    if _NC_CACHE is None or _NC_CACHE[4] != wids:
        wh = _whash(inputs)
        if _NC_CACHE is None or _NC_CACHE[0] != wh:
            nc, _ = build_nc(**inputs)
            in_maps = make_core_inputs(**inputs)
            run, args, order = make_runner(nc, in_maps)
            _NC_CACHE = (wh, nc, (run, order), args, wids)
        else:
            _NC_CACHE = (_NC_CACHE[0], _NC_CACHE[1], _NC_CACHE[2],
                         _NC_CACHE[3], wids)
    _, nc, (run, order), _, _ = _NC_CACHE
    in_maps = make_core_inputs(**inputs)
    args = [jax.device_put(np.concatenate(
        [np.asarray(in_maps[c][nm]) for c in range(NCORES)], axis=0))
        for nm in order]
    out = run(*args)
    out_np = [np.asarray(o) for o in out]
    return assemble_output(out_np)


if __name__ == "__main__":
    rng = np.random.default_rng(0)
    nc, _ = build_nc(
        Wq=rng.standard_normal((H * DH, D), np.float32) * 0.02,
        Wkv=rng.standard_normal((2 * H * DH, D), np.float32) * 0.02,
        q_scale=np.ones(DH, np.float32), k_scale=np.ones(DH, np.float32),
        Wg=rng.standard_normal((H, D), np.float32) * 0.02,
        bg=np.zeros(H, np.float32),
        Wo=rng.standard_normal((D, H * DH), np.float32) * 0.02)
    print("built ok")
